# revision 7
# baseline (speedup 1.0000x reference)
"""Trainium2 Bass kernel for a 3-layer GCN (JKNet, mode='cat') — 8-core SPMD.

Strategy (dst-sharded graph parallelism, v4):
  - Nodes are partitioned across 8 cores (6250 each, padded to 6272 = 49*128).
    Each core owns all edges whose destination lands in its range.
  - Canonical per-core node order posA is a TWO-SEGMENT degree sort: positions
    [0, 3200) hold the core's "first-half" nodes (local index < 3200), the
    rest at [3200, 6272). Each half of every core's h@W slice is AllGathered
    separately (table1 / table2, both < 32768 rows for int16 dma_gather
    indices), and the collectives are software-pipelined: the tail for acc
    slots 0-24 runs right after the seg1 gather stream, so AG1 of the NEXT
    layer overlaps this layer's seg2 stream, and AG2 overlaps the next
    layer's sys1 stream.
  - System-1 (sources with position < 3200) accumulates in its own full
    degree-sort order (posB) and is reconciled into the canonical accumulator
    through a DRAM bounce + permute-gather. System-2 (table half 2)
    accumulates directly in posA order as two independent sub-streams (one
    per posA segment, each with dense rounds).
  - The whole feature path runs in bf16 (2x DVE throughput): table rows are
    128-wide bf16 (256B, the dma_gather granularity; top half is padding),
    accumulators/messages/weights bf16, PSUM stays f32. Round 0 of each
    stream is written with a direct multiply (no memset + add).
  - Gathers use 2048-index dma_gather instructions; dynamic_dma_scratch_size
    is raised to 64KB so the SWDGE descriptor rings absorb a whole
    instruction without stalling the Pool engine. Queue 0 is avoided while a
    collective is in flight (they contend).
  - PSUM->SBUF copies and the fused bias+ReLU (applied post-transpose where
    bias is per-partition) run on the Scalar/ACT engine. PE transposes are
    paired (two 64-wide slots per stationary load).

Self-contained: hardcodes the problem geometry (N=50000, E=800000, 128->64,
3 layers, out 40) but computes all data-dependent schedules from the inputs.
"""

import sys

sys.path.insert(0, "/opt/trn_rl_repo")

import numpy as np
from ml_dtypes import bfloat16

N = 50000
E = 800000
IN_DIM = 128
HID = 64
OUT_DIM = 40
M = 8               # cores
NPC = N // M        # 6250 nodes per core
SLOTS = 49          # ceil(6250/128)
SLICE = SLOTS * 128  # 6272 padded rows per core slice
S1 = 3200           # canonical positions < S1 go to table half 1
S1_SLOTS = S1 // 128          # 25
S2 = SLICE - S1                # 3072
S2_SLOTS = S2 // 128          # 24
T1_ROWS = M * S1    # 25600 (< 32768, int16-safe)
T2_ROWS = M * S2    # 24576
ROW = 128           # bf16 table row width (256B, gather granularity)
CMAX = 2048         # max indices per dma_gather instruction
NQ = 4              # SWDGE queues


def _wrap16(a):
    """Flat [L] -> [128, L//16] int16, index j at partition j%16, slot j//16,
    replicated across the 8 GPSIMD core groups."""
    L = a.shape[0]
    return np.tile(a.reshape(L // 16, 16).T, (8, 1)).astype(np.int16)


def _wrap128(a):
    """Flat [L] -> [128, L//128], position j at partition j%128, slot j//128."""
    L = a.shape[0]
    return np.ascontiguousarray(a.reshape(L // 128, 128).T)


def _rowof(q, slots):
    """acc position q (within a segment) -> wrapped DRAM row (partition-major
    layout of a [128, slots, ...] staging tile)."""
    return (q % 128) * slots + q // 128


def _ranks_within(p):
    """For int array p, rank of each element among equal values (stable)."""
    order = np.argsort(p, kind="stable")
    ps = p[order]
    starts = np.r_[0, np.nonzero(np.diff(ps))[0] + 1]
    counts = np.diff(np.r_[starts, len(ps)])
    r_sorted = np.arange(len(ps)) - np.repeat(starts, counts)
    r = np.empty_like(r_sorted)
    r[order] = r_sorted
    return r


def _pad128(n):
    return ((int(n) + 127) // 128) * 128


def _chunk_plan(widths, base_slot, r0_end):
    """Chunks for one stream whose rounds are dense ranges starting at acc
    slot base_slot. Returns (L, [(off, w, [(msg_slot, acc_slot, n, direct)])]).
    """
    roundoff = np.r_[0, np.cumsum(widths)].astype(np.int64)
    L = int(roundoff[-1])
    chunks = []
    off = 0
    while off < L:
        w = min(CMAX, L - off)
        segs = []
        a = off
        s = int(np.searchsorted(roundoff, a, side="right")) - 1
        while a < off + w:
            b = min(off + w, int(roundoff[s + 1]))
            if a < r0_end < b:
                b = r0_end
            segs.append(((a - off) // 128,
                         base_slot + int(a - roundoff[s]) // 128,
                         (b - a) // 128, a < r0_end))
            a = b
            if a >= roundoff[s + 1]:
                s += 1
        chunks.append((off, w, segs))
        off += w
    return L, chunks


def _prep(x, edge_index, edge_weight):
    """All host-side index prep. Returns (plan dict, per-core input maps)."""
    src = np.asarray(edge_index[0], dtype=np.int64)
    dst = np.asarray(edge_index[1], dtype=np.int64)
    ew = np.asarray(edge_weight, dtype=np.float32)
    x = np.asarray(x, dtype=np.float32)

    dcore = dst // NPC
    dloc = dst - dcore * NPC
    sloc = src - (src // NPC) * NPC
    # System split by the SOURCE's local index: sys1 sources sit in table1.
    is1 = sloc < S1

    # Per-core sorts. posA: two-segment sort by sys2-degree (canonical/table
    # order). posB: full sort by sys1-degree (bounced accumulator order).
    posA = np.empty(N, np.int64)
    posB = np.empty(N, np.int64)
    piA_all = []
    deg2a_sorted, deg2b_sorted, deg1_sorted = [], [], []
    for c in range(M):
        mask = dcore == c
        l1 = dloc[mask & is1]     # sys1 edges' local dst
        l2 = dloc[mask & ~is1]    # sys2 edges' local dst
        deg1 = np.bincount(l1, minlength=NPC)
        deg2 = np.bincount(l2, minlength=NPC)
        o1 = np.arange(S1)[np.argsort(-deg2[:S1], kind="stable")]
        o2 = np.arange(S1, NPC)[np.argsort(-deg2[S1:], kind="stable")]
        piA = np.concatenate([o1, o2])      # position -> local node
        piB = np.argsort(-deg1, kind="stable")
        pA = np.empty(NPC, np.int64); pA[piA] = np.arange(NPC)
        pB = np.empty(NPC, np.int64); pB[piB] = np.arange(NPC)
        posA[c * NPC:(c + 1) * NPC] = pA
        posB[c * NPC:(c + 1) * NPC] = pB
        piA_all.append(piA)
        deg2a_sorted.append(deg2[o1])
        deg2b_sorted.append(deg2[o2])
        deg1_sorted.append(deg1[piB])

    # Global round widths (shared across cores -> max over cores).
    def widths_of(deg_sorted_list):
        smax = max((int(d[0]) if len(d) else 0) for d in deg_sorted_list)
        out = []
        for s in range(smax):
            n_s = max(int((d > s).sum()) for d in deg_sorted_list)
            if n_s == 0:
                break
            out.append(_pad128(n_s))
        return out

    w1 = widths_of(deg1_sorted)      # sys1 (single dense segment, posB)
    w2a = widths_of(deg2a_sorted)    # sys2 stream a (posA seg1, slots 0-24)
    w2b = widths_of(deg2b_sorted)    # sys2 stream b (posA seg2, slots 25-48)

    L1, chunks1 = _chunk_plan(w1, 0, w1[0])
    L2a, chunks2a = _chunk_plan(w2a, 0, w2a[0])
    L2b, chunks2b = _chunk_plan(w2b, S1_SLOTS, w2b[0])
    off1 = np.r_[0, np.cumsum(w1)].astype(np.int64)
    off2a = np.r_[0, np.cumsum(w2a)].astype(np.int64)
    off2b = np.r_[0, np.cumsum(w2b)].astype(np.int64)

    # Accumulator slot ranges never covered by round 0 (need zeroing).
    z1 = (w1[0] // 128, SLOTS)
    zA = []
    if w2a[0] < S1:
        zA.append((w2a[0] // 128, S1_SLOTS))
    if w2b[0] < S2:
        zA.append((S1_SLOTS + w2b[0] // 128, SLOTS))

    # Gather table row of a node (as seen from any core).
    ocore = np.arange(N) // NPC
    t1row = ocore * S1 + _rowof(posA, S1_SLOTS)              # valid if pos < S1
    q2 = posA - S1
    t2row = ocore * S2 + _rowof(np.maximum(q2, 0), S2_SLOTS)  # valid if pos >= S1

    in_maps = []
    for c in range(M):
        mask = dcore == c
        m1 = mask & is1
        m2 = mask & ~is1

        # sys1 flat arrays (posB order, table1 indices)
        d1 = dst[m1]
        r1 = _ranks_within(posB[d1])
        flat1 = off1[r1] + posB[d1]
        idx1 = np.zeros(L1, np.int64)
        ew1 = np.zeros(L1, np.float32)
        idx1[flat1] = t1row[src[m1]]
        ew1[flat1] = ew[m1]
        assert idx1.max(initial=0) < T1_ROWS

        # sys2 split into two sub-streams by the DST's posA segment
        d2 = dst[m2]
        qd = posA[d2]
        in_a = qd < S1
        idx2a = np.zeros(L2a, np.int64); ew2a = np.zeros(L2a, np.float32)
        idx2b = np.zeros(L2b, np.int64); ew2b = np.zeros(L2b, np.float32)
        for sel, offs, idxo, ewo, qoff in (
                (in_a, off2a, idx2a, ew2a, 0),
                (~in_a, off2b, idx2b, ew2b, S1)):
            dd = d2[sel]
            rr = _ranks_within(posA[dd])
            flat = offs[rr] + posA[dd] - qoff
            idxo[flat] = t2row[src[m2][sel]]
            ewo[flat] = ew[m2][sel]
        assert idx2a.max(initial=0) < T2_ROWS
        assert idx2b.max(initial=0) < T2_ROWS

        # permute map: canonical position q -> bounce row of the same node's
        # posB position. Pad positions point at an always-zero row.
        piA = piA_all[c]
        rho = np.full(SLICE, NPC, np.int64)
        rho[:NPC] = posB[c * NPC + piA]
        rho_rows = _rowof(rho, SLOTS)

        # x slice, transposed, in canonical order (pad columns zero)
        xT = np.zeros((IN_DIM, SLICE), np.float32)
        xT[:, :NPC] = x[c * NPC + piA, :].T

        in_maps.append({
            "xT": xT.astype(bfloat16),
            "idx1": _wrap16(idx1), "ew1": _wrap128(ew1).astype(bfloat16),
            "idx2a": _wrap16(idx2a), "ew2a": _wrap128(ew2a).astype(bfloat16),
            "idx2b": _wrap16(idx2b), "ew2b": _wrap128(ew2b).astype(bfloat16),
            "rho": _wrap16(rho_rows),
        })

    plan = {
        "L1": L1, "L2a": L2a, "L2b": L2b,
        "chunks1": chunks1, "chunks2a": chunks2a, "chunks2b": chunks2b,
        "z1": z1, "zA": zA,
        "posA": posA,
    }
    return plan, in_maps


def _build(plan, W1, b1, W2, b2, W3, b3, Wlin, blin):
    import concourse.bacc as bacc
    import concourse.mybir as mybir
    import concourse.tile as tile

    L1, L2a, L2b = plan["L1"], plan["L2a"], plan["L2b"]
    f32 = mybir.dt.float32
    bf16 = mybir.dt.bfloat16
    i16 = mybir.dt.int16

    nc = bacc.Bacc("TRN2", target_bir_lowering=False, debug=False,
                   num_devices=M, num_swdge_queues=NQ,
                   dynamic_dma_scratch_size=65536)

    # ---- I/O ----
    xT_d = nc.dram_tensor("xT", [IN_DIM, SLICE], bf16, kind="ExternalInput")
    idx1_d = nc.dram_tensor("idx1", [128, L1 // 16], i16, kind="ExternalInput")
    ew1_d = nc.dram_tensor("ew1", [128, L1 // 128], bf16, kind="ExternalInput")
    idx2a_d = nc.dram_tensor("idx2a", [128, L2a // 16], i16, kind="ExternalInput")
    ew2a_d = nc.dram_tensor("ew2a", [128, L2a // 128], bf16, kind="ExternalInput")
    idx2b_d = nc.dram_tensor("idx2b", [128, L2b // 16], i16, kind="ExternalInput")
    ew2b_d = nc.dram_tensor("ew2b", [128, L2b // 128], bf16, kind="ExternalInput")
    rho_d = nc.dram_tensor("rho", [128, SLICE // 16], i16, kind="ExternalInput")
    W1_d = nc.dram_tensor("W1", [IN_DIM, HID], bf16, kind="ExternalInput")
    W2_d = nc.dram_tensor("W2", [HID, HID], bf16, kind="ExternalInput")
    W3_d = nc.dram_tensor("W3", [128, HID], bf16, kind="ExternalInput")  # rows 64-127 hold W3
    Wl12_d = nc.dram_tensor("Wl12", [128, OUT_DIM], bf16, kind="ExternalInput")
    Wl3_d = nc.dram_tensor("Wl3", [HID, OUT_DIM], bf16, kind="ExternalInput")
    bias_d = nc.dram_tensor("bias", [128, 3], f32, kind="ExternalInput")
    blin_d = nc.dram_tensor("blin", [128, OUT_DIM], f32, kind="ExternalInput")
    out_d = nc.dram_tensor("out", [128, SLOTS, OUT_DIM], f32, kind="ExternalOutput")

    # internal DRAM (table rows are 128-wide bf16 = 256B, the gather granule)
    slice1_d = nc.dram_tensor("slice1_hw", [128, S1_SLOTS, ROW], bf16)
    slice2_d = nc.dram_tensor("slice2_hw", [128, S2_SLOTS, ROW], bf16)
    table1_d = nc.dram_tensor("table1", [T1_ROWS, ROW], bf16, addr_space="Shared")
    table2_d = nc.dram_tensor("table2", [T2_ROWS, ROW], bf16, addr_space="Shared")
    bounce_d = nc.dram_tensor("bounce", [SLICE, ROW], bf16)

    # per-phase queue cycling; `allowed` avoids queue 0 while a collective
    # is in flight (they contend on the DMA rings)
    qstate = [0]

    def nextq(allowed=(0, 1, 2, 3)):
        q = allowed[qstate[0] % len(allowed)]
        qstate[0] += 1
        return q

    with tile.TileContext(nc) as tc:
        with (
            tc.tile_pool(name="const", bufs=1) as constp,
            tc.tile_pool(name="acc", bufs=1) as accp,
            tc.tile_pool(name="ht", bufs=1) as htp,
            tc.tile_pool(name="stag", bufs=1) as stagp,
            tc.tile_pool(name="msg", bufs=6) as msgp,
            tc.tile_pool(name="ps", bufs=3, space="PSUM") as psp,
            tc.tile_pool(name="pst", bufs=3, space="PSUM") as pstp,
            tc.tile_pool(name="pso", bufs=2, space="PSUM") as psop,
        ):
            # ---- load constants ----
            xT = constp.tile([IN_DIM, SLICE], bf16)
            idx1 = constp.tile([128, L1 // 16], i16)
            ew1 = constp.tile([128, L1 // 128], bf16)
            idx2a = constp.tile([128, L2a // 16], i16)
            ew2a = constp.tile([128, L2a // 128], bf16)
            idx2b = constp.tile([128, L2b // 16], i16)
            ew2b = constp.tile([128, L2b // 128], bf16)
            rho = constp.tile([128, SLICE // 16], i16)
            W1t = constp.tile([IN_DIM, HID], bf16)
            W2t = constp.tile([HID, HID], bf16)
            W3t = constp.tile([128, HID], bf16)  # W3 lives in partitions 64-127
            Wl12t = constp.tile([128, OUT_DIM], bf16)
            Wl3t = constp.tile([HID, OUT_DIM], bf16)
            biast = constp.tile([128, 3], f32)
            blint = constp.tile([128, OUT_DIM], f32)
            ident = constp.tile([128, 128], f32)
            identb = constp.tile([128, 128], bf16)

            for t, d in ((xT, xT_d), (idx1, idx1_d), (ew1, ew1_d),
                         (idx2a, idx2a_d), (ew2a, ew2a_d),
                         (idx2b, idx2b_d), (ew2b, ew2b_d), (rho, rho_d),
                         (W1t, W1_d), (W2t, W2_d), (Wl12t, Wl12_d),
                         (Wl3t, Wl3_d), (biast, bias_d), (blint, blin_d)):
                nc.sync.dma_start(t[:], d[:])
            nc.sync.dma_start(W3t[:], W3_d[:])
            from concourse.masks import make_identity
            make_identity(nc, ident[:])
            nc.scalar.copy(identb[:], ident[:])

            h12T = htp.tile([128, SLICE], bf16)   # rows 0-63: h1^T, 64-127: h2^T
            h3T = htp.tile([HID, SLICE], bf16)

            relu = mybir.ActivationFunctionType.Relu

            def ag1():
                nc.gpsimd.collective_compute(
                    "AllGather", mybir.AluOpType.bypass,
                    replica_groups=[list(range(M))],
                    ins=[slice1_d[:]], outs=[table1_d[:]])

            def ag2():
                nc.gpsimd.collective_compute(
                    "AllGather", mybir.AluOpType.bypass,
                    replica_groups=[list(range(M))],
                    ins=[slice2_d[:]], outs=[table2_d[:]])

            # ---- layer-1 input matmuls: slice of x @ W1 ----
            stag = stagp.tile([128, SLOTS, ROW], bf16, tag="stag")
            for m in range(SLOTS):
                ps = psp.tile([128, HID], f32, tag="mm")
                nc.tensor.matmul(ps[:], xT[:, m * 128:(m + 1) * 128], W1t[:],
                                 start=True, stop=True)
                nc.scalar.copy(stag[:, m, 0:HID], ps[:])
                if m == S1_SLOTS - 1:
                    nc.sync.dma_start(slice1_d[:], stag[:, :S1_SLOTS, :])
            nc.sync.dma_start(slice2_d[:], stag[:, S1_SLOTS:, :])
            ag1()
            ag2()

            ostag = stagp.tile([128, SLOTS, OUT_DIM], f32, tag="ostag")

            def emit_chunks(acc, idx_t, ew_t, chunks, tbl, allowed,
                            after=None):
                """after: {chunk_index: callable} emitted mid-stream."""
                for i, (off, w, segs) in enumerate(chunks):
                    ws = w // 128
                    msg = msgp.tile([128, CMAX // 128, ROW], bf16, tag="msg")
                    nc.gpsimd.dma_gather(
                        msg[:, :ws, :], tbl, idx_t[:, off // 16:(off + w) // 16],
                        w, w, ROW, single_packet=False,
                        queue_num=nextq(allowed))
                    for (ms, as_, ns, direct) in segs:
                        ewb = (ew_t[:, (off + ms * 128) // 128:
                                    (off + (ms + ns) * 128) // 128]
                               .to_broadcast([128, ns, HID]))
                        if direct:
                            # round 0: write acc = msg * ew (no memset+add)
                            nc.vector.tensor_mul(
                                acc[:, as_:as_ + ns, 0:HID],
                                msg[:, ms:ms + ns, 0:HID], ewb)
                        else:
                            nc.vector.tensor_mul(
                                msg[:, ms:ms + ns, 0:HID],
                                msg[:, ms:ms + ns, 0:HID], ewb)
                            nc.vector.tensor_add(
                                acc[:, as_:as_ + ns, 0:HID],
                                acc[:, as_:as_ + ns, 0:HID],
                                msg[:, ms:ms + ns, 0:HID])
                    if after and i in after:
                        after[i]()

            def emit_fold(accA, lo_slot, hi_slot, allowed):
                """Gather bounce rows for acc slots [lo, hi) and fold into
                accA (canonical order)."""
                for off in range(lo_slot * 128, hi_slot * 128, CMAX):
                    w = min(CMAX, hi_slot * 128 - off)
                    ws = w // 128
                    msg = msgp.tile([128, CMAX // 128, ROW], bf16, tag="msg")
                    nc.gpsimd.dma_gather(
                        msg[:, :ws, :], bounce_d[:],
                        rho[:, off // 16:(off + w) // 16],
                        w, w, ROW, single_packet=False,
                        queue_num=nextq(allowed))
                    nc.vector.tensor_add(
                        accA[:, off // 128:off // 128 + ws, :],
                        accA[:, off // 128:off // 128 + ws, :],
                        msg[:, :ws, 0:HID])

            for layer in range(3):
                accA = accp.tile([128, SLOTS, HID], bf16, tag="accA")
                accB = accp.tile([128, SLOTS, ROW], bf16, tag="accB")
                # zero only slots round 0 doesn't cover (usually almost none)
                s0, s1_ = plan["z1"]
                if s0 < s1_:
                    nc.scalar.memzero(accB[:, s0:s1_, :])
                for (a0, a1) in plan["zA"]:
                    nc.scalar.memzero(accA[:, a0:a1, :])
                if layer > 0:
                    # warmup gathers (queues 1-3) while AG2 is in flight, so
                    # post-collective DGE state reload happens off the
                    # critical path (results are discarded)
                    for q in (1, 2, 3):
                        wmsg = msgp.tile([128, CMAX // 128, ROW], bf16, tag="msg")
                        nc.gpsimd.dma_gather(
                            wmsg[:, :1, :], bounce_d[:], rho[:, 0:8],
                            128, 128, ROW, single_packet=False, queue_num=q)

                # sys1 (table half 1, posB order) -> accB.
                # AG2 of THIS layer is in flight during the early chunks
                # (steady state), so stay off queue 0.
                emit_chunks(accB, idx1, ew1, plan["chunks1"], table1_d[:],
                            allowed=(1, 2, 3))
                # queue-0 warmup once the collective window has passed
                wmsg = msgp.tile([128, CMAX // 128, ROW], bf16, tag="msg")
                nc.gpsimd.dma_gather(
                    wmsg[:, :1, :], bounce_d[:], rho[:, 0:8],
                    128, 128, ROW, single_packet=False, queue_num=0)
                nc.sync.dma_start(
                    bounce_d[:].rearrange("(p s) d -> p s d", p=128), accB[:])

                # sys2 stream a (acc slots 0-24, collective-free window)
                emit_chunks(accA, idx2a, ew2a, plan["chunks2a"], table2_d[:],
                            allowed=(0, 1, 2, 3))
                emit_fold(accA, 0, S1_SLOTS, allowed=(0, 1, 2, 3))

                # ---- tail half 1 (slots 0-24): transpose, bias+relu, matmul,
                # stage; then slice1 write and NEXT layer's AG1 (overlapping
                # the seg2 stream below).
                if layer < 2:
                    stag = stagp.tile([128, SLOTS, ROW], bf16, tag="stag")

                def tail_group(mlo, mhi):
                    for m in range(mlo, mhi, 2):
                        npair = min(2, mhi - m)
                        pst = pstp.tile([128, 128], bf16, tag="tr")
                        nc.tensor.transpose(
                            pst[:npair * 64, :], accA[:, m:m + npair, :],
                            identb[:])
                        for j in range(npair):
                            sl = slice((m + j) * 128, (m + j + 1) * 128)
                            bcol = biast[j * 64:(j + 1) * 64, layer:layer + 1]
                            if layer == 0:
                                nc.scalar.activation(
                                    h12T[0:HID, sl], pst[j * 64:(j + 1) * 64, :],
                                    relu, bias=bcol)
                            elif layer == 1:
                                nc.scalar.activation(
                                    h12T[HID:128, sl], pst[j * 64:(j + 1) * 64, :],
                                    relu, bias=bcol)
                            else:
                                nc.scalar.activation(
                                    h3T[:, sl], pst[j * 64:(j + 1) * 64, :],
                                    relu, bias=bcol)
                    for m in range(mlo, mhi):
                        sl = slice(m * 128, (m + 1) * 128)
                        if layer == 0:
                            ps = psp.tile([128, HID], f32, tag="mm")
                            nc.tensor.matmul(ps[:], h12T[0:HID, sl], W2t[:],
                                             start=True, stop=True)
                            nc.scalar.copy(stag[:, m, 0:HID], ps[:])
                        elif layer == 1:
                            ps = psp.tile([128, HID], f32, tag="mm")
                            nc.tensor.matmul(ps[:], h12T[HID:128, sl],
                                             W3t[HID:128, :],
                                             start=True, stop=True)
                            nc.scalar.copy(stag[:, m, 0:HID], ps[:])
                        else:
                            pso = psop.tile([128, OUT_DIM], f32, tag="out")
                            nc.tensor.matmul(pso[:], h12T[:, sl],
                                             Wl12t[:], start=True, stop=False)
                            nc.tensor.matmul(pso[:], h3T[:, sl],
                                             Wl3t[:], start=False, stop=True)
                            nc.vector.tensor_add(ostag[:, m, :], pso[:], blint[:])

                tail_group(0, S1_SLOTS)
                if layer < 2:
                    nc.sync.dma_start(slice1_d[:], stag[:, :S1_SLOTS, :])

                # sys2 stream b (acc slots 25-48) with next layer's AG1
                # issued a few chunks in (so its input-dep wait is covered,
                # and the collective overlaps the rest of the stream).
                after = {2: ag1} if layer < 2 else None
                emit_chunks(accA, idx2b, ew2b, plan["chunks2b"], table2_d[:],
                            allowed=(1, 2, 3), after=after)
                emit_fold(accA, S1_SLOTS, SLOTS, allowed=(1, 2, 3))

                tail_group(S1_SLOTS, SLOTS)
                if layer < 2:
                    nc.sync.dma_start(slice2_d[:], stag[:, S1_SLOTS:, :])
                    ag2()

            nc.sync.dma_start(out_d[:], ostag[:])

    nc.compile()
    return nc


_CACHE = {}


def kernel(x, edge_index, edge_weight, W1, b1, W2, b2, W3, b3, Wlin, blin):
    from concourse.bass_utils import run_bass_kernel_spmd

    x = np.asarray(x, dtype=np.float32)
    assert x.shape == (N, IN_DIM) and np.asarray(edge_index).shape == (2, E)

    key = hash(np.asarray(edge_index).tobytes())
    if key not in _CACHE:
        plan, in_maps = _prep(x, edge_index, edge_weight)
        nc = _build(plan, W1, b1, W2, b2, W3, b3, Wlin, blin)
        _CACHE[key] = (plan, nc)
    else:
        plan, nc = _CACHE[key]
        _, in_maps = _prep(x, edge_index, edge_weight)

    Wlin = np.asarray(Wlin, dtype=np.float32)
    bias_col = np.zeros((128, 3), np.float32)
    for l, b in enumerate((b1, b2, b3)):
        b = np.asarray(b, np.float32)
        bias_col[0:HID, l] = b
        bias_col[HID:128, l] = b
    bf = lambda a: np.ascontiguousarray(np.asarray(a, np.float32)).astype(bfloat16)
    shared = {
        "W1": bf(W1), "W2": bf(W2),
        "W3": bf(np.concatenate([np.zeros((HID, HID), np.float32),
                                 np.asarray(W3, np.float32)], axis=0)),
        "Wl12": bf(Wlin[0:128]), "Wl3": bf(Wlin[128:192]),
        "bias": bias_col,
        "blin": np.tile(np.asarray(blin, np.float32)[None, :], (128, 1)),
    }
    for im in in_maps:
        im.update(shared)

    res = run_bass_kernel_spmd(nc, in_maps, core_ids=list(range(M)))
    kernel._last_results = res
    kernel._last_in_maps = in_maps
    kernel._last_nc = nc

    posA = plan["posA"]
    out = np.empty((N, OUT_DIM), np.float32)
    for c in range(M):
        oc = res.results[c]["out"]  # [128, SLOTS, OUT]
        q = posA[c * NPC:(c + 1) * NPC]
        out[c * NPC:(c + 1) * NPC] = oc[q % 128, q // 128, :]
    return out


# revision 8
# speedup vs baseline: 1.1177x; 1.1177x over previous
"""Trainium2 Bass kernel for a 3-layer GCN (JKNet, mode='cat') — 8-core SPMD.

Strategy (dst-sharded graph parallelism, v4):
  - Nodes are partitioned across 8 cores (6250 each, padded to 6272 = 49*128).
    Each core owns all edges whose destination lands in its range.
  - Canonical per-core node order posA is a TWO-SEGMENT degree sort: positions
    [0, 3200) hold the core's "first-half" nodes (local index < 3200), the
    rest at [3200, 6272). Each half of every core's h@W slice is AllGathered
    separately (table1 / table2, both < 32768 rows for int16 dma_gather
    indices), and the collectives are software-pipelined: the tail for acc
    slots 0-24 runs right after the seg1 gather stream, so AG1 of the NEXT
    layer overlaps this layer's seg2 stream, and AG2 overlaps the next
    layer's sys1 stream.
  - System-1 (sources with position < 3200) accumulates in its own full
    degree-sort order (posB) and is reconciled into the canonical accumulator
    through a DRAM bounce + permute-gather. System-2 (table half 2)
    accumulates directly in posA order as two independent sub-streams (one
    per posA segment, each with dense rounds).
  - The whole feature path runs in bf16 (2x DVE throughput): table rows are
    128-wide bf16 (256B, the dma_gather granularity; top half is padding),
    accumulators/messages/weights bf16, PSUM stays f32. Round 0 of each
    stream is written with a direct multiply (no memset + add).
  - Gathers use 2048-index dma_gather instructions; dynamic_dma_scratch_size
    is raised to 64KB so the SWDGE descriptor rings absorb a whole
    instruction without stalling the Pool engine. Queue 0 is avoided while a
    collective is in flight (they contend).
  - PSUM->SBUF copies and the fused bias+ReLU (applied post-transpose where
    bias is per-partition) run on the Scalar/ACT engine. PE transposes are
    paired (two 64-wide slots per stationary load).

Self-contained: hardcodes the problem geometry (N=50000, E=800000, 128->64,
3 layers, out 40) but computes all data-dependent schedules from the inputs.
"""

import sys

sys.path.insert(0, "/opt/trn_rl_repo")

import numpy as np
from ml_dtypes import bfloat16

N = 50000
E = 800000
IN_DIM = 128
HID = 64
OUT_DIM = 40
M = 8               # cores
NPC = N // M        # 6250 nodes per core
SLOTS = 49          # ceil(6250/128)
SLICE = SLOTS * 128  # 6272 padded rows per core slice
S1 = 3200           # canonical positions < S1 go to table half 1
S1_SLOTS = S1 // 128          # 25
S2 = SLICE - S1                # 3072
S2_SLOTS = S2 // 128          # 24
T1_ROWS = M * S1    # 25600 (< 32768, int16-safe)
T2_ROWS = M * S2    # 24576
ROW = 128           # bf16 table row width (256B, gather granularity)
CMAX = 2048         # max indices per dma_gather instruction
NQ = 4              # SWDGE queues


def _wrap16(a):
    """Flat [L] -> [128, L//16] int16, index j at partition j%16, slot j//16,
    replicated across the 8 GPSIMD core groups."""
    L = a.shape[0]
    return np.tile(a.reshape(L // 16, 16).T, (8, 1)).astype(np.int16)


def _wrap128(a):
    """Flat [L] -> [128, L//128], position j at partition j%128, slot j//128."""
    L = a.shape[0]
    return np.ascontiguousarray(a.reshape(L // 128, 128).T)


def _rowof(q, slots):
    """acc position q (within a segment) -> wrapped DRAM row (partition-major
    layout of a [128, slots, ...] staging tile)."""
    return (q % 128) * slots + q // 128


def _ranks_within(p):
    """For int array p, rank of each element among equal values (stable)."""
    order = np.argsort(p, kind="stable")
    ps = p[order]
    starts = np.r_[0, np.nonzero(np.diff(ps))[0] + 1]
    counts = np.diff(np.r_[starts, len(ps)])
    r_sorted = np.arange(len(ps)) - np.repeat(starts, counts)
    r = np.empty_like(r_sorted)
    r[order] = r_sorted
    return r


def _pad128(n):
    return ((int(n) + 127) // 128) * 128


def _chunk_plan(widths, base_slot, r0_end):
    """Chunks for one stream whose rounds are dense ranges starting at acc
    slot base_slot. Returns (L, [(off, w, [(msg_slot, acc_slot, n, direct)])]).
    """
    roundoff = np.r_[0, np.cumsum(widths)].astype(np.int64)
    L = int(roundoff[-1])
    chunks = []
    off = 0
    while off < L:
        w = min(CMAX, L - off)
        segs = []
        a = off
        s = int(np.searchsorted(roundoff, a, side="right")) - 1
        while a < off + w:
            b = min(off + w, int(roundoff[s + 1]))
            if a < r0_end < b:
                b = r0_end
            segs.append(((a - off) // 128,
                         base_slot + int(a - roundoff[s]) // 128,
                         (b - a) // 128, a < r0_end))
            a = b
            if a >= roundoff[s + 1]:
                s += 1
        chunks.append((off, w, segs))
        off += w
    return L, chunks


def _prep(x, edge_index, edge_weight):
    """All host-side index prep. Returns (plan dict, per-core input maps)."""
    src = np.asarray(edge_index[0], dtype=np.int64)
    dst = np.asarray(edge_index[1], dtype=np.int64)
    ew = np.asarray(edge_weight, dtype=np.float32)
    x = np.asarray(x, dtype=np.float32)

    dcore = dst // NPC
    dloc = dst - dcore * NPC
    sloc = src - (src // NPC) * NPC
    # System split by the SOURCE's local index: sys1 sources sit in table1.
    is1 = sloc < S1

    # Per-core sorts. posA: two-segment sort by sys2-degree (canonical/table
    # order). posB: full sort by sys1-degree (bounced accumulator order).
    posA = np.empty(N, np.int64)
    posB = np.empty(N, np.int64)
    piA_all = []
    deg2a_sorted, deg2b_sorted, deg1_sorted = [], [], []
    for c in range(M):
        mask = dcore == c
        l1 = dloc[mask & is1]     # sys1 edges' local dst
        l2 = dloc[mask & ~is1]    # sys2 edges' local dst
        deg1 = np.bincount(l1, minlength=NPC)
        deg2 = np.bincount(l2, minlength=NPC)
        o1 = np.arange(S1)[np.argsort(-deg2[:S1], kind="stable")]
        o2 = np.arange(S1, NPC)[np.argsort(-deg2[S1:], kind="stable")]
        piA = np.concatenate([o1, o2])      # position -> local node
        piB = np.argsort(-deg1, kind="stable")
        pA = np.empty(NPC, np.int64); pA[piA] = np.arange(NPC)
        pB = np.empty(NPC, np.int64); pB[piB] = np.arange(NPC)
        posA[c * NPC:(c + 1) * NPC] = pA
        posB[c * NPC:(c + 1) * NPC] = pB
        piA_all.append(piA)
        deg2a_sorted.append(deg2[o1])
        deg2b_sorted.append(deg2[o2])
        deg1_sorted.append(deg1[piB])

    # Global round widths (shared across cores -> max over cores).
    def widths_of(deg_sorted_list):
        smax = max((int(d[0]) if len(d) else 0) for d in deg_sorted_list)
        out = []
        for s in range(smax):
            n_s = max(int((d > s).sum()) for d in deg_sorted_list)
            if n_s == 0:
                break
            out.append(_pad128(n_s))
        return out

    w1 = widths_of(deg1_sorted)      # sys1 (single dense segment, posB)
    w2a = widths_of(deg2a_sorted)    # sys2 stream a (posA seg1, slots 0-24)
    w2b = widths_of(deg2b_sorted)    # sys2 stream b (posA seg2, slots 25-48)

    L1, chunks1 = _chunk_plan(w1, 0, w1[0])
    L2a, chunks2a = _chunk_plan(w2a, 0, w2a[0])
    L2b, chunks2b = _chunk_plan(w2b, S1_SLOTS, w2b[0])
    off1 = np.r_[0, np.cumsum(w1)].astype(np.int64)
    off2a = np.r_[0, np.cumsum(w2a)].astype(np.int64)
    off2b = np.r_[0, np.cumsum(w2b)].astype(np.int64)

    # Accumulator slot ranges never covered by round 0 (need zeroing).
    z1 = (w1[0] // 128, SLOTS)
    zA = []
    if w2a[0] < S1:
        zA.append((w2a[0] // 128, S1_SLOTS))
    if w2b[0] < S2:
        zA.append((S1_SLOTS + w2b[0] // 128, SLOTS))

    # Gather table row of a node (as seen from any core).
    ocore = np.arange(N) // NPC
    t1row = ocore * S1 + _rowof(posA, S1_SLOTS)              # valid if pos < S1
    q2 = posA - S1
    t2row = ocore * S2 + _rowof(np.maximum(q2, 0), S2_SLOTS)  # valid if pos >= S1

    in_maps = []
    for c in range(M):
        mask = dcore == c
        m1 = mask & is1
        m2 = mask & ~is1

        # sys1 flat arrays (posB order, table1 indices)
        d1 = dst[m1]
        r1 = _ranks_within(posB[d1])
        flat1 = off1[r1] + posB[d1]
        idx1 = np.zeros(L1, np.int64)
        ew1 = np.zeros(L1, np.float32)
        idx1[flat1] = t1row[src[m1]]
        ew1[flat1] = ew[m1]
        assert idx1.max(initial=0) < T1_ROWS

        # sys2 split into two sub-streams by the DST's posA segment
        d2 = dst[m2]
        qd = posA[d2]
        in_a = qd < S1
        idx2a = np.zeros(L2a, np.int64); ew2a = np.zeros(L2a, np.float32)
        idx2b = np.zeros(L2b, np.int64); ew2b = np.zeros(L2b, np.float32)
        for sel, offs, idxo, ewo, qoff in (
                (in_a, off2a, idx2a, ew2a, 0),
                (~in_a, off2b, idx2b, ew2b, S1)):
            dd = d2[sel]
            rr = _ranks_within(posA[dd])
            flat = offs[rr] + posA[dd] - qoff
            idxo[flat] = t2row[src[m2][sel]]
            ewo[flat] = ew[m2][sel]
        assert idx2a.max(initial=0) < T2_ROWS
        assert idx2b.max(initial=0) < T2_ROWS

        # permute map: canonical position q -> bounce row of the same node's
        # posB position. Pad positions point at an always-zero row.
        piA = piA_all[c]
        rho = np.full(SLICE, NPC, np.int64)
        rho[:NPC] = posB[c * NPC + piA]
        rho_rows = _rowof(rho, SLOTS)

        # x slice, transposed, in canonical order (pad columns zero)
        xT = np.zeros((IN_DIM, SLICE), np.float32)
        xT[:, :NPC] = x[c * NPC + piA, :].T

        in_maps.append({
            "xT": xT.astype(bfloat16),
            "idx1": _wrap16(idx1), "ew1": _wrap128(ew1).astype(bfloat16),
            "idx2a": _wrap16(idx2a), "ew2a": _wrap128(ew2a).astype(bfloat16),
            "idx2b": _wrap16(idx2b), "ew2b": _wrap128(ew2b).astype(bfloat16),
            "rho": _wrap16(rho_rows),
        })

    plan = {
        "L1": L1, "L2a": L2a, "L2b": L2b,
        "chunks1": chunks1, "chunks2a": chunks2a, "chunks2b": chunks2b,
        "z1": z1, "zA": zA,
        "posA": posA,
    }
    return plan, in_maps


def _build(plan, W1, b1, W2, b2, W3, b3, Wlin, blin):
    import concourse.bacc as bacc
    import concourse.mybir as mybir
    import concourse.tile as tile

    L1, L2a, L2b = plan["L1"], plan["L2a"], plan["L2b"]
    f32 = mybir.dt.float32
    bf16 = mybir.dt.bfloat16
    i16 = mybir.dt.int16

    nc = bacc.Bacc("TRN2", target_bir_lowering=False, debug=False,
                   num_devices=M, num_swdge_queues=NQ,
                   dynamic_dma_scratch_size=65536)

    # ---- I/O ----
    xT_d = nc.dram_tensor("xT", [IN_DIM, SLICE], bf16, kind="ExternalInput")
    idx1_d = nc.dram_tensor("idx1", [128, L1 // 16], i16, kind="ExternalInput")
    ew1_d = nc.dram_tensor("ew1", [128, L1 // 128], bf16, kind="ExternalInput")
    idx2a_d = nc.dram_tensor("idx2a", [128, L2a // 16], i16, kind="ExternalInput")
    ew2a_d = nc.dram_tensor("ew2a", [128, L2a // 128], bf16, kind="ExternalInput")
    idx2b_d = nc.dram_tensor("idx2b", [128, L2b // 16], i16, kind="ExternalInput")
    ew2b_d = nc.dram_tensor("ew2b", [128, L2b // 128], bf16, kind="ExternalInput")
    rho_d = nc.dram_tensor("rho", [128, SLICE // 16], i16, kind="ExternalInput")
    W1_d = nc.dram_tensor("W1", [IN_DIM, HID], bf16, kind="ExternalInput")
    W2_d = nc.dram_tensor("W2", [HID, HID], bf16, kind="ExternalInput")
    W3_d = nc.dram_tensor("W3", [128, HID], bf16, kind="ExternalInput")  # rows 64-127 hold W3
    Wl12_d = nc.dram_tensor("Wl12", [128, OUT_DIM], bf16, kind="ExternalInput")
    Wl3_d = nc.dram_tensor("Wl3", [HID, OUT_DIM], bf16, kind="ExternalInput")
    bias_d = nc.dram_tensor("bias", [128, 3], f32, kind="ExternalInput")
    blin_d = nc.dram_tensor("blin", [128, OUT_DIM], f32, kind="ExternalInput")
    out_d = nc.dram_tensor("out", [128, SLOTS, OUT_DIM], f32, kind="ExternalOutput")

    # internal DRAM (table rows are 128-wide bf16 = 256B, the gather granule)
    slice1_d = nc.dram_tensor("slice1_hw", [128, S1_SLOTS, ROW], bf16)
    slice2_d = nc.dram_tensor("slice2_hw", [128, S2_SLOTS, ROW], bf16)
    table1_d = nc.dram_tensor("table1", [T1_ROWS, ROW], bf16, addr_space="Shared")
    table2_d = nc.dram_tensor("table2", [T2_ROWS, ROW], bf16, addr_space="Shared")
    bounce_d = nc.dram_tensor("bounce", [SLICE, ROW], bf16)

    # per-phase queue cycling; `allowed` avoids queue 0 while a collective
    # is in flight (they contend on the DMA rings)
    qstate = [0]

    def nextq(allowed=(0, 1, 2, 3)):
        q = allowed[qstate[0] % len(allowed)]
        qstate[0] += 1
        return q

    with tile.TileContext(nc) as tc:
        with (
            tc.tile_pool(name="const", bufs=1) as constp,
            tc.tile_pool(name="acc", bufs=1) as accp,
            tc.tile_pool(name="ht", bufs=1) as htp,
            tc.tile_pool(name="stag", bufs=1) as stagp,
            tc.tile_pool(name="msg", bufs=8) as msgp,
            tc.tile_pool(name="ps", bufs=3, space="PSUM") as psp,
            tc.tile_pool(name="pst", bufs=3, space="PSUM") as pstp,
            tc.tile_pool(name="pso", bufs=2, space="PSUM") as psop,
        ):
            # ---- load constants ----
            xT = constp.tile([IN_DIM, SLICE], bf16)
            idx1 = constp.tile([128, L1 // 16], i16)
            ew1 = constp.tile([128, L1 // 128], bf16)
            idx2a = constp.tile([128, L2a // 16], i16)
            ew2a = constp.tile([128, L2a // 128], bf16)
            idx2b = constp.tile([128, L2b // 16], i16)
            ew2b = constp.tile([128, L2b // 128], bf16)
            rho = constp.tile([128, SLICE // 16], i16)
            W1t = constp.tile([IN_DIM, HID], bf16)
            W2t = constp.tile([HID, HID], bf16)
            W3t = constp.tile([128, HID], bf16)  # W3 lives in partitions 64-127
            Wl12t = constp.tile([128, OUT_DIM], bf16)
            Wl3t = constp.tile([HID, OUT_DIM], bf16)
            biast = constp.tile([128, 3], f32)
            blint = constp.tile([128, OUT_DIM], f32)
            ident = constp.tile([128, 128], f32)
            identb = constp.tile([128, 128], bf16)

            for t, d in ((xT, xT_d), (idx1, idx1_d), (ew1, ew1_d),
                         (idx2a, idx2a_d), (ew2a, ew2a_d),
                         (idx2b, idx2b_d), (ew2b, ew2b_d), (rho, rho_d),
                         (W1t, W1_d), (W2t, W2_d), (Wl12t, Wl12_d),
                         (Wl3t, Wl3_d), (biast, bias_d), (blint, blin_d)):
                nc.sync.dma_start(t[:], d[:])
            nc.sync.dma_start(W3t[:], W3_d[:])
            from concourse.masks import make_identity
            make_identity(nc, ident[:])
            nc.scalar.copy(identb[:], ident[:])

            h12T = htp.tile([128, SLICE], bf16)   # rows 0-63: h1^T, 64-127: h2^T
            h3T = htp.tile([HID, SLICE], bf16)

            relu = mybir.ActivationFunctionType.Relu

            def ag1():
                nc.gpsimd.collective_compute(
                    "AllGather", mybir.AluOpType.bypass,
                    replica_groups=[list(range(M))],
                    ins=[slice1_d[:]], outs=[table1_d[:]])

            def ag2():
                nc.gpsimd.collective_compute(
                    "AllGather", mybir.AluOpType.bypass,
                    replica_groups=[list(range(M))],
                    ins=[slice2_d[:]], outs=[table2_d[:]])

            # ---- layer-1 input matmuls: slice of x @ W1 ----
            stag = stagp.tile([128, SLOTS, ROW], bf16, tag="stag")
            for m in range(SLOTS):
                ps = psp.tile([128, HID], f32, tag="mm")
                nc.tensor.matmul(ps[:], xT[:, m * 128:(m + 1) * 128], W1t[:],
                                 start=True, stop=True)
                nc.scalar.copy(stag[:, m, 0:HID], ps[:])
                if m == S1_SLOTS - 1:
                    nc.sync.dma_start(slice1_d[:], stag[:, :S1_SLOTS, :])
            nc.sync.dma_start(slice2_d[:], stag[:, S1_SLOTS:, :])
            ag1()
            ag2()

            ostag = stagp.tile([128, SLOTS, OUT_DIM], f32, tag="ostag")

            def emit_chunks(acc, idx_t, ew_t, chunks, tbl, allowed,
                            after=None):
                """after: {chunk_index: callable} emitted mid-stream."""
                for i, (off, w, segs) in enumerate(chunks):
                    ws = w // 128
                    msg = msgp.tile([128, CMAX // 128, ROW], bf16, tag="msg")
                    nc.gpsimd.dma_gather(
                        msg[:, :ws, :], tbl, idx_t[:, off // 16:(off + w) // 16],
                        w, w, ROW, single_packet=False,
                        queue_num=nextq(allowed))
                    for (ms, as_, ns, direct) in segs:
                        ewb = (ew_t[:, (off + ms * 128) // 128:
                                    (off + (ms + ns) * 128) // 128]
                               .to_broadcast([128, ns, HID]))
                        if direct:
                            # round 0: write acc = msg * ew (no memset+add)
                            nc.vector.tensor_mul(
                                acc[:, as_:as_ + ns, 0:HID],
                                msg[:, ms:ms + ns, 0:HID], ewb)
                        else:
                            nc.vector.tensor_mul(
                                msg[:, ms:ms + ns, 0:HID],
                                msg[:, ms:ms + ns, 0:HID], ewb)
                            nc.vector.tensor_add(
                                acc[:, as_:as_ + ns, 0:HID],
                                acc[:, as_:as_ + ns, 0:HID],
                                msg[:, ms:ms + ns, 0:HID])
                    if after and i in after:
                        after[i]()

            def emit_fold(accA, lo_slot, hi_slot, allowed):
                """Gather bounce rows for acc slots [lo, hi) and fold into
                accA (canonical order)."""
                for off in range(lo_slot * 128, hi_slot * 128, CMAX):
                    w = min(CMAX, hi_slot * 128 - off)
                    ws = w // 128
                    msg = msgp.tile([128, CMAX // 128, ROW], bf16, tag="msg")
                    nc.gpsimd.dma_gather(
                        msg[:, :ws, :], bounce_d[:],
                        rho[:, off // 16:(off + w) // 16],
                        w, w, ROW, single_packet=False,
                        queue_num=nextq(allowed))
                    nc.vector.tensor_add(
                        accA[:, off // 128:off // 128 + ws, :],
                        accA[:, off // 128:off // 128 + ws, :],
                        msg[:, :ws, 0:HID])

            for layer in range(3):
                accA = accp.tile([128, SLOTS, HID], bf16, tag="accA")
                accB = accp.tile([128, SLOTS, ROW], bf16, tag="accB")
                # zero only slots round 0 doesn't cover (usually almost none)
                s0, s1_ = plan["z1"]
                if s0 < s1_:
                    nc.scalar.memzero(accB[:, s0:s1_, :])
                for (a0, a1) in plan["zA"]:
                    nc.scalar.memzero(accA[:, a0:a1, :])
                if layer > 0:
                    # warmup gathers (queues 1-3) while AG2 is in flight, so
                    # post-collective DGE state reload happens off the
                    # critical path (results are discarded)
                    for q in (0, 1, 2, 3):
                        wmsg = msgp.tile([128, CMAX // 128, ROW], bf16, tag="msg")
                        nc.gpsimd.dma_gather(
                            wmsg[:, :1, :], bounce_d[:], rho[:, 0:8],
                            128, 128, ROW, single_packet=False, queue_num=q)

                # sys1 (table half 1, posB order) -> accB.
                # AG2 of THIS layer is in flight during the early chunks
                # (steady state), so stay off queue 0.
                emit_chunks(accB, idx1, ew1, plan["chunks1"], table1_d[:],
                            allowed=(0, 1, 2, 3))
                nc.sync.dma_start(
                    bounce_d[:].rearrange("(p s) d -> p s d", p=128), accB[:])

                # sys2 stream a (acc slots 0-24, collective-free window)
                emit_chunks(accA, idx2a, ew2a, plan["chunks2a"], table2_d[:],
                            allowed=(0, 1, 2, 3))
                emit_fold(accA, 0, S1_SLOTS, allowed=(0, 1, 2, 3))

                # ---- tail half 1 (slots 0-24): transpose, bias+relu, matmul,
                # stage; then slice1 write and NEXT layer's AG1 (overlapping
                # the seg2 stream below).
                if layer < 2:
                    stag = stagp.tile([128, SLOTS, ROW], bf16, tag="stag")

                def tail_group(mlo, mhi):
                    for m in range(mlo, mhi, 2):
                        npair = min(2, mhi - m)
                        pst = pstp.tile([128, 128], bf16, tag="tr")
                        nc.tensor.transpose(
                            pst[:npair * 64, :], accA[:, m:m + npair, :],
                            identb[:])
                        for j in range(npair):
                            sl = slice((m + j) * 128, (m + j + 1) * 128)
                            bcol = biast[j * 64:(j + 1) * 64, layer:layer + 1]
                            if layer == 0:
                                nc.scalar.activation(
                                    h12T[0:HID, sl], pst[j * 64:(j + 1) * 64, :],
                                    relu, bias=bcol)
                            elif layer == 1:
                                nc.scalar.activation(
                                    h12T[HID:128, sl], pst[j * 64:(j + 1) * 64, :],
                                    relu, bias=bcol)
                            else:
                                nc.scalar.activation(
                                    h3T[:, sl], pst[j * 64:(j + 1) * 64, :],
                                    relu, bias=bcol)
                    for m in range(mlo, mhi):
                        sl = slice(m * 128, (m + 1) * 128)
                        if layer == 0:
                            ps = psp.tile([128, HID], f32, tag="mm")
                            nc.tensor.matmul(ps[:], h12T[0:HID, sl], W2t[:],
                                             start=True, stop=True)
                            nc.scalar.copy(stag[:, m, 0:HID], ps[:])
                        elif layer == 1:
                            ps = psp.tile([128, HID], f32, tag="mm")
                            nc.tensor.matmul(ps[:], h12T[HID:128, sl],
                                             W3t[HID:128, :],
                                             start=True, stop=True)
                            nc.scalar.copy(stag[:, m, 0:HID], ps[:])
                        else:
                            pso = psop.tile([128, OUT_DIM], f32, tag="out")
                            nc.tensor.matmul(pso[:], h12T[:, sl],
                                             Wl12t[:], start=True, stop=False)
                            nc.tensor.matmul(pso[:], h3T[:, sl],
                                             Wl3t[:], start=False, stop=True)
                            nc.vector.tensor_add(ostag[:, m, :], pso[:], blint[:])

                tail_group(0, S1_SLOTS)
                if layer < 2:
                    nc.sync.dma_start(slice1_d[:], stag[:, :S1_SLOTS, :])

                # sys2 stream b (acc slots 25-48) with next layer's AG1
                # issued a few chunks in (so its input-dep wait is covered,
                # and the collective overlaps the rest of the stream).
                after = {2: ag1} if layer < 2 else None
                emit_chunks(accA, idx2b, ew2b, plan["chunks2b"], table2_d[:],
                            allowed=(0, 1, 2, 3), after=after)
                emit_fold(accA, S1_SLOTS, SLOTS, allowed=(0, 1, 2, 3))

                tail_group(S1_SLOTS, SLOTS)
                if layer < 2:
                    nc.sync.dma_start(slice2_d[:], stag[:, S1_SLOTS:, :])
                    ag2()

            nc.sync.dma_start(out_d[:], ostag[:])

    nc.compile()
    return nc


_CACHE = {}


def kernel(x, edge_index, edge_weight, W1, b1, W2, b2, W3, b3, Wlin, blin):
    from concourse.bass_utils import run_bass_kernel_spmd

    x = np.asarray(x, dtype=np.float32)
    assert x.shape == (N, IN_DIM) and np.asarray(edge_index).shape == (2, E)

    key = hash(np.asarray(edge_index).tobytes())
    if key not in _CACHE:
        plan, in_maps = _prep(x, edge_index, edge_weight)
        nc = _build(plan, W1, b1, W2, b2, W3, b3, Wlin, blin)
        _CACHE[key] = (plan, nc)
    else:
        plan, nc = _CACHE[key]
        _, in_maps = _prep(x, edge_index, edge_weight)

    Wlin = np.asarray(Wlin, dtype=np.float32)
    bias_col = np.zeros((128, 3), np.float32)
    for l, b in enumerate((b1, b2, b3)):
        b = np.asarray(b, np.float32)
        bias_col[0:HID, l] = b
        bias_col[HID:128, l] = b
    bf = lambda a: np.ascontiguousarray(np.asarray(a, np.float32)).astype(bfloat16)
    shared = {
        "W1": bf(W1), "W2": bf(W2),
        "W3": bf(np.concatenate([np.zeros((HID, HID), np.float32),
                                 np.asarray(W3, np.float32)], axis=0)),
        "Wl12": bf(Wlin[0:128]), "Wl3": bf(Wlin[128:192]),
        "bias": bias_col,
        "blin": np.tile(np.asarray(blin, np.float32)[None, :], (128, 1)),
    }
    for im in in_maps:
        im.update(shared)

    res = run_bass_kernel_spmd(nc, in_maps, core_ids=list(range(M)))
    kernel._last_results = res
    kernel._last_in_maps = in_maps
    kernel._last_nc = nc

    posA = plan["posA"]
    out = np.empty((N, OUT_DIM), np.float32)
    for c in range(M):
        oc = res.results[c]["out"]  # [128, SLOTS, OUT]
        q = posA[c * NPC:(c + 1) * NPC]
        out[c * NPC:(c + 1) * NPC] = oc[q % 128, q // 128, :]
    return out
